# revision 44
# baseline (speedup 1.0000x reference)
"""Causal self-attention (GQA + RoPE) Trainium2 Bass kernel, 8 NeuronCores.

Sharding: 2-way data parallel over batch x 4-way tensor parallel over heads.
Core c handles batch c//4 and query heads [4*(c%4), 4*(c%4)+4) plus the one
KV head g = c%4 that serves them (n_kv_heads=4 -> no KV replication).
Each core computes a partial [S, D] output (its heads' slice of the out
projection); the host sums the 4 partials per batch.

Device layouts are transposed ("feature-major"): x is loaded pre-transposed;
projections produce qT/kT [dim, tokens]; attention scores are computed as
S^T = kT.T @ qT.  The P@V contraction is TOKEN-major: per 128-token block,
matmul(out[128t,129], lhsT=e_block[128k,128t], rhs=[v|ones][128k,129]) - the
129th column accumulates the softmax denominator for free, so there are no
separate row-sum matmuls and normalization is a per-partition (per-token)
scale.  The normalized [token, dim] tiles are DMA-transposed back to
[dim, token] for the output projection.  RoPE is handled by de-interleaving
the q/k weight rows on the host so rotation pairs become (p, p+64) partition
pairs.  TensorEngine-facing tensors are bf16 (fp32 PSUM accumulation).

The PE stream is kept dense by interleaving projection / out-projection
matmul groups as fillers inside the (Scalar-bound) attention kb loop.
"""

import sys

if "/opt/trn_rl_repo" not in sys.path:
    sys.path.insert(0, "/opt/trn_rl_repo")

import math

import numpy as np

D_MODEL = 2048
N_HEADS = 16
N_KV_HEADS = 4
ROPE_THETA = 10000.0
B, S = 2, 2048
DK = D_MODEL // N_HEADS          # 128
NCORES = 8
NEG = -1e30

_COMPILED = None
_TRACE = False                   # test.py flips this for profiling runs
_LAST_RESULT = None              # BassKernelResults of the last run


def _build():
    import concourse.bacc as bacc
    import concourse.tile as tile
    from concourse import mybir

    f32 = mybir.dt.float32
    bf16 = mybir.dt.bfloat16

    nc = bacc.Bacc("TRN2", debug=False, target_bir_lowering=False)

    def inp(name, shape, dt=bf16):
        return nc.declare_dram_parameter(name, list(shape), dt, isOutput=False).ap()

    x_d = inp("x", [128, 16, S])
    wq_d = inp("wq", [128, 4, 16, 128])      # m-major: [p, m, db, 128]
    wkv_d = inp("wkv", [128, 16, 256])
    wc_d = inp("wc", [128, 4, 2048])
    cos_d = inp("cos2", [128, S], f32)
    sin_d = inp("ss", [128, S], f32)
    dmask_d = inp("dmask", [128, 128])       # bf16, applied via PE matmul
    ident_d = inp("ident", [128, 128])
    out_d = nc.declare_dram_parameter("out", [S, D_MODEL], f32, isOutput=True).ap()

    EXP = mybir.ActivationFunctionType.Exp
    COPY = mybir.ActivationFunctionType.Copy

    with tile.TileContext(nc) as tc:
        with (
            tc.tile_pool(name="consts", bufs=1) as consts,
            tc.tile_pool(name="qpool", bufs=4) as qpool,
            tc.tile_pool(name="vch", bufs=2) as vchp,
            tc.tile_pool(name="tmp", bufs=2) as tmpp,
            tc.tile_pool(name="epool", bufs=8) as epool,
            tc.tile_pool(name="rpool", bufs=8) as rpool,
            tc.tile_pool(name="otcp", bufs=2) as otcp,
            tc.tile_pool(name="otTp", bufs=2) as otTp,
            tc.tile_pool(name="osb", bufs=4) as osbp,
            tc.tile_pool(name="psum_st", bufs=2, space="PSUM") as psum_st,
            tc.tile_pool(name="psum_av", bufs=4, space="PSUM") as psum_av,
        ):
            # ---- constants / weights ----
            wq_sb = consts.tile([128, 4, 16, 128], bf16, tag="wq")
            wkv_sb = consts.tile([128, 16, 256], bf16, tag="wkv")
            wc_sb = consts.tile([128, 4, 2048], bf16, tag="wc")
            c2_sb = consts.tile([128, S], f32, tag="cos2")
            ss_sb = consts.tile([128, S], f32, tag="ss")
            dmask_sb = consts.tile([128, 128], bf16, tag="dmask")
            ident_sb = consts.tile([128, 128], bf16, tag="ident")
            kTr_sb = consts.tile([128, S], bf16, tag="kTr")
            v_sb = consts.tile([128, 16, 256], bf16, tag="V")
            xT = consts.tile([128, 16, S], bf16, tag="xT")

            # input loads ordered by first use: attention goes chunk 3 -> 0,
            # so x arrives by token range (high chunks first) split across
            # the sync + vector queues; weights lead on the scalar queue and
            # the rope tables ride the gpsimd (SWDGE) queue.
            nc.scalar.dma_start(out=wq_sb[:, 0, :, :], in_=wq_d[:, 0, :, :])
            for c in (3, 2, 1, 0):
                t0 = c * 512
                nc.sync.dma_start(
                    out=xT[:, 0:8, t0:t0 + 512], in_=x_d[:, 0:8, t0:t0 + 512]
                )
                nc.gpsimd.dma_start(
                    out=xT[:, 8:16, t0:t0 + 512], in_=x_d[:, 8:16, t0:t0 + 512]
                )
                if c == 3:
                    nc.gpsimd.dma_start(out=c2_sb, in_=cos_d)
                    nc.gpsimd.dma_start(out=ss_sb, in_=sin_d)
            nc.scalar.dma_start(out=wkv_sb, in_=wkv_d)
            nc.scalar.dma_start(out=dmask_sb, in_=dmask_d)
            nc.scalar.dma_start(out=ident_sb, in_=ident_d)
            nc.scalar.dma_start(out=wq_sb[:, 1:4, :, :], in_=wq_d[:, 1:4, :, :])
            nc.gpsimd.memset(v_sb[:, :, 128:129], 1.0)

            def rope(dst, src, c):
                """dst[128,512] (bf16 SBUF) <- rotate(src[128,512] f32 PSUM).

                Row p<64 holds the even (te) element of pair p, row p+64 the
                odd (to): dst_lo = te*cos - to*sin; dst_hi = to*cos + te*sin.
                """
                cs = c2_sb[:, c * 512:(c + 1) * 512]
                sn = ss_sb[:, c * 512:(c + 1) * 512]
                t = tmpp.tile([128, 512], f32, tag="ropesin")
                t2 = tmpp.tile([128, 512], f32, tag="ropecos")
                nc.vector.tensor_mul(t[0:64, :], src[64:128, :], sn[0:64, :])
                nc.vector.tensor_mul(t[64:128, :], src[0:64, :], sn[64:128, :])
                nc.vector.tensor_mul(t2, src, cs)
                nc.vector.tensor_add(dst, t2, t)

            qTrs = {}
            proj_pool_cm = tc.tile_pool(name="psum_proj", bufs=2, space="PSUM")
            psum = proj_pool_cm.__enter__()

            def gen_qproj(c, ms=(0, 1, 2, 3)):
                """Generator: emits Q projection for chunk c, yielding every
                2 accumulation steps (~2x512-row matmuls) for interleaving."""
                tq0 = c * 512
                if c in qTrs:
                    qTr = qTrs[c]
                else:
                    qTr = qpool.tile([128, 4, 512], bf16, tag="qTr", name=f"qTr{c}")
                    qTrs[c] = qTr
                for m in ms:
                    pq = psum.tile([128, 512], f32, tag="mm512")
                    for db in range(16):
                        nc.tensor.matmul(
                            pq,
                            lhsT=wq_sb[:, m, db, :],
                            rhs=xT[:, db, tq0:tq0 + 512],
                            start=(db == 0),
                            stop=(db == 15),
                        )
                        if db % 2 == 1:
                            yield
                    rope(qTr[:, m, :], pq, c)

            def gen_kvproj(c):
                tq0 = c * 512
                pk = psum.tile([128, 512], f32, tag="mm512")
                for db in range(16):
                    nc.tensor.matmul(
                        pk,
                        lhsT=wkv_sb[:, db, 0:128],
                        rhs=xT[:, db, tq0:tq0 + 512],
                        start=(db == 0),
                        stop=(db == 15),
                    )
                    if db % 2 == 1:
                        yield
                rope(kTr_sb[:, tq0:tq0 + 512], pk, c)
                pv = psum.tile([128, 512], f32, tag="mm512")
                for db in range(16):
                    nc.tensor.matmul(
                        pv,
                        lhsT=wkv_sb[:, db, 128:256],
                        rhs=xT[:, db, tq0:tq0 + 512],
                        start=(db == 0),
                        stop=(db == 15),
                    )
                    if db % 2 == 1:
                        yield
                vch = vchp.tile([128, 512], bf16, tag="vch")
                nc.scalar.copy(out=vch, in_=pv)
                # per-block 2D DMA transposes: strided 3D-out DMAT corrupts
                # data in the XBAR path, so keep each out contiguous
                for rr in range(4):
                    nc.sync.dma_start_transpose(
                        out=v_sb[:, 4 * c + rr, 0:128],
                        in_=vch[:, rr * 128:(rr + 1) * 128],
                    )

            def run_gen(g):
                for _ in g:
                    pass

            # ---- projection prologue: everything attention chunk 3 needs ----
            run_gen(gen_qproj(3, ms=(0,)))
            run_gen(gen_kvproj(3))
            run_gen(gen_qproj(3, ms=(1, 2, 3)))
            run_gen(gen_kvproj(2))
            run_gen(gen_kvproj(1))
            run_gen(gen_kvproj(0))
            nc.scalar.dma_start(out=wc_sb, in_=wc_d)

            # ---- fillers: remaining projections, then out-projections ----
            out_pool_cm = [None]
            psum_o = [None]

            def gen_outproj(tq0, otT):
                """Generator: out-projection for chunk at tq0 from otT
                [128d, 4h, 512t]; yields after each psum tile (4 matmuls)."""
                for tb in range(4):
                    row = tq0 + tb * 128
                    for oc in range(4):
                        po = psum_o[0].tile([128, 512], f32, tag="out")
                        for h in range(4):
                            nc.tensor.matmul(
                                po,
                                lhsT=otT[:, h, tb, :],
                                rhs=wc_sb[:, h, oc * 512:(oc + 1) * 512],
                                start=(h == 0),
                                stop=(h == 3),
                            )
                        osb = osbp.tile([128, 512], f32, tag="osb")
                        if oc % 2 == 0:
                            nc.vector.tensor_copy(out=osb, in_=po)
                        else:
                            nc.scalar.copy(out=osb, in_=po)
                        nc.sync.dma_start(
                            out=out_d[row:row + 128, oc * 512:(oc + 1) * 512],
                            in_=osb,
                        )
                        yield

            proj_gens = [gen_qproj(2), gen_qproj(1), gen_qproj(0)]
            out_gens = []

            def pull_filler(n=1):
                """Emit up to n filler steps: projections first, then
                out-projections (swapping PSUM pools at the boundary)."""
                while n > 0:
                    if proj_gens:
                        try:
                            next(proj_gens[0])
                            n -= 1
                        except StopIteration:
                            proj_gens.pop(0)
                            if not proj_gens:
                                # all projections done: swap proj psum pool
                                # for the out-projection pool (2 banks each)
                                proj_pool_cm.__exit__(None, None, None)
                                cm = tc.tile_pool(
                                    name="psum_o", bufs=2, space="PSUM"
                                )
                                out_pool_cm[0] = cm
                                psum_o[0] = cm.__enter__()
                    elif out_gens:
                        try:
                            next(out_gens[0])
                            n -= 1
                        except StopIteration:
                            out_gens.pop(0)
                    else:
                        return

            # ---- attention chunks, biggest first ----
            pending_outproj = []
            for c in (3, 2, 1, 0):
                tq0 = c * 512
                qTr = qTrs[c]
                nkb = 4 * c + 4
                otT_c = otTp.tile([128, 4, 4, 128], bf16, tag="otT", name=f"otT{c}")
                for h in range(4):
                    if h == 1 and pending_outproj:
                        # release the previous chunk's out-projection only
                        # after one head of slack so its transposes land
                        out_gens.append(pending_outproj.pop(0))
                    otc = otcp.tile([128, 4, 128], bf16, tag="otc")
                    pav = [
                        psum_av.tile([128, 129], f32, tag="pav", name=f"pav{tb}")
                        for tb in range(4)
                    ]

                    def st_mm(kb):
                        """Score matmul for one key block (cols col0:512).

                        Diagonal blocks get the causal mask accumulated as a
                        second PE matmul (identity.T @ dmask == dmask), which
                        keeps the scores->exp chain entirely on the PE.
                        """
                        rr = kb - 4 * c
                        col0 = 0 if rr < 0 else 128 * rr
                        pst = psum_st.tile([128, 512], f32, tag="st")
                        nc.tensor.matmul(
                            pst[:, col0:512],
                            lhsT=kTr_sb[:, kb * 128:(kb + 1) * 128],
                            rhs=qTr[:, h, col0:512],
                            start=True,
                            stop=(rr < 0),
                            skip_group_check=True,
                        )
                        if rr >= 0:
                            nc.tensor.matmul(
                                pst[:, col0:col0 + 128],
                                lhsT=ident_sb,
                                rhs=dmask_sb,
                                start=False,
                                stop=True,
                                skip_group_check=True,
                            )
                        return pst, col0

                    # software-pipelined: emit S^T(kb+1) ahead of the
                    # exp-dependent AV matmuls of kb so the PE stream never
                    # head-blocks on the activation engine
                    pending = st_mm(0)
                    for kb in range(nkb):
                        pst, col0 = pending
                        if kb + 1 < nkb:
                            pending = st_mm(kb + 1)
                        e = epool.tile([128, 512], bf16, tag="E")
                        nc.scalar.activation(
                            out=e[:, col0:512], in_=pst[:, col0:512], func=EXP
                        )
                        rr = kb - 4 * c
                        tb_lo = rr if rr > 0 else 0
                        for tb in range(tb_lo, 4):
                            nc.tensor.matmul(
                                pav[tb],
                                lhsT=e[:, tb * 128:(tb + 1) * 128],
                                rhs=v_sb[:, kb, 0:129],
                                start=(kb == 0),
                                stop=(kb == 4 * c + tb),
                            )
                        pull_filler(1)

                    # normalize: per-token (per-partition) reciprocal of the
                    # accumulated 129th column, then one blocked
                    # DMA-transpose [t,d] -> [d,t] for the out-proj
                    for tb in range(4):
                        r = rpool.tile([128, 1], f32, tag="r")
                        nc.vector.reciprocal_approx_fast(
                            out=r, in_=pav[tb][:, 128:129]
                        )
                        if tb % 2 == 0:
                            nc.vector.tensor_scalar_mul(
                                otc[:, tb, :], pav[tb][:, 0:128], r
                            )
                        else:
                            nc.scalar.activation(
                                out=otc[:, tb, :], in_=pav[tb][:, 0:128],
                                func=COPY, scale=r,
                            )
                    nc.sync.dma_start_transpose(
                        out=otT_c[:, h, :, :], in_=otc[:, :, :]
                    )
                    pull_filler(2)

                # queue this chunk's out-projection as filler work (released
                # one head into the next chunk's attention)
                pending_outproj.append(gen_outproj(tq0, otT_c))
            while pending_outproj:
                out_gens.append(pending_outproj.pop(0))

            # drain remaining fillers (last chunk's outproj + leftovers)
            pull_filler(10**9)
            if out_pool_cm[0] is not None:
                out_pool_cm[0].__exit__(None, None, None)

    nc.compile()
    return nc


def _host_prep(x, Wq, Wkv, Wc):
    """Shard + relayout the full inputs into the 8 per-core input dicts."""
    import ml_dtypes

    bf = ml_dtypes.bfloat16
    dk, H, KV = DK, N_HEADS, N_KV_HEADS
    x = np.asarray(x, np.float32)
    Wq = np.asarray(Wq, np.float32)
    Wkv = np.asarray(Wkv, np.float32)
    Wc = np.asarray(Wc, np.float32)

    p = np.concatenate([np.arange(0, dk, 2), np.arange(1, dk, 2)])
    perm_q = np.concatenate([h * dk + p for h in range(H)])
    Wq_p = (Wq / math.sqrt(dk))[perm_q]
    perm_k = np.concatenate([g * dk + p for g in range(KV)])
    Wk_p = Wkv[:KV * dk][perm_k]
    Wv = Wkv[KV * dk:]

    pairs = np.arange(dk // 2, dtype=np.float64)
    freqs = 1.0 / (ROPE_THETA ** (2.0 * pairs / dk))
    ang = np.arange(S, dtype=np.float64)[:, None] * freqs[None, :]
    cos_t = np.cos(ang).astype(np.float32).T  # [64, S]
    sin_t = np.sin(ang).astype(np.float32).T
    c2 = np.ascontiguousarray(np.concatenate([cos_t, cos_t], 0))   # [128, S]
    ss = np.ascontiguousarray(np.concatenate([-sin_t, sin_t], 0))  # [128, S]

    jj = np.arange(128)[None, :]
    pp = np.arange(128)[:, None]
    dmask = np.where(pp <= jj, 0.0, NEG).astype(bf)
    ident = np.eye(128, dtype=bf)

    maps = []
    for core in range(NCORES):
        b, g = core // 4, core % 4
        # [p, m, db, 128]: feature-partition p, head m, contraction block db
        wq_l = np.ascontiguousarray(
            Wq_p[512 * g:512 * g + 512].T
            .reshape(16, 128, 4, 128).transpose(1, 2, 0, 3)
        ).astype(bf)
        wkv_sl = np.concatenate(
            [Wk_p[g * dk:(g + 1) * dk], Wv[g * dk:(g + 1) * dk]], 0
        ).T  # [2048, 256]
        wkv_l = np.ascontiguousarray(
            wkv_sl.reshape(16, 128, 256).transpose(1, 0, 2)
        ).astype(bf)
        wc_l = np.ascontiguousarray(
            Wc[:, 512 * g:512 * g + 512].T.reshape(4, 128, 2048).transpose(1, 0, 2)
        ).astype(bf)
        xt_l = np.ascontiguousarray(
            x[b].T.reshape(16, 128, S).transpose(1, 0, 2)
        ).astype(bf)
        maps.append(dict(
            x=xt_l, wq=wq_l, wkv=wkv_l, wc=wc_l,
            cos2=c2, ss=ss, dmask=dmask, ident=ident,
        ))
    return maps


def kernel(x, Wq, Wkv, Wc):
    global _COMPILED, _LAST_RESULT
    from concourse.bass_utils import run_bass_kernel_spmd

    if _COMPILED is None:
        _COMPILED = _build()
    in_maps = _host_prep(x, Wq, Wkv, Wc)
    res = run_bass_kernel_spmd(
        _COMPILED, in_maps, core_ids=list(range(NCORES)), trace=_TRACE
    )
    _LAST_RESULT = res
    outs = [res.results[i]["out"] for i in range(NCORES)]
    full = np.stack(
        [outs[0] + outs[1] + outs[2] + outs[3],
         outs[4] + outs[5] + outs[6] + outs[7]], 0
    ).astype(np.float32)
    return full


# revision 46
# speedup vs baseline: 1.0024x; 1.0024x over previous
"""Causal self-attention (GQA + RoPE) Trainium2 Bass kernel, 8 NeuronCores.

Sharding: 2-way data parallel over batch x 4-way tensor parallel over heads.
Core c handles batch c//4 and query heads [4*(c%4), 4*(c%4)+4) plus the one
KV head g = c%4 that serves them (n_kv_heads=4 -> no KV replication).
Each core computes a partial [S, D] output (its heads' slice of the out
projection); the host sums the 4 partials per batch.

Device layouts are transposed ("feature-major"): x is loaded pre-transposed;
projections produce qT/kT [dim, tokens]; attention scores are computed as
S^T = kT.T @ qT.  The P@V contraction is TOKEN-major: per 128-token block,
matmul(out[128t,129], lhsT=e_block[128k,128t], rhs=[v|ones][128k,129]) - the
129th column accumulates the softmax denominator for free, so there are no
separate row-sum matmuls and normalization is a per-partition (per-token)
scale.  The normalized [token, dim] tiles are DMA-transposed back to
[dim, token] for the output projection.  RoPE is handled by de-interleaving
the q/k weight rows on the host so rotation pairs become (p, p+64) partition
pairs.  TensorEngine-facing tensors are bf16 (fp32 PSUM accumulation).

The PE stream is kept dense by interleaving projection / out-projection
matmul groups as fillers inside the (Scalar-bound) attention kb loop.
"""

import sys

if "/opt/trn_rl_repo" not in sys.path:
    sys.path.insert(0, "/opt/trn_rl_repo")

import math

import numpy as np

D_MODEL = 2048
N_HEADS = 16
N_KV_HEADS = 4
ROPE_THETA = 10000.0
B, S = 2, 2048
DK = D_MODEL // N_HEADS          # 128
NCORES = 8
NEG = -1e30

_COMPILED = None
_TRACE = False                   # test.py flips this for profiling runs
_LAST_RESULT = None              # BassKernelResults of the last run


def _build():
    import concourse.bacc as bacc
    import concourse.tile as tile
    from concourse import mybir

    f32 = mybir.dt.float32
    bf16 = mybir.dt.bfloat16

    nc = bacc.Bacc("TRN2", debug=False, target_bir_lowering=False)

    def inp(name, shape, dt=bf16):
        return nc.declare_dram_parameter(name, list(shape), dt, isOutput=False).ap()

    x_d = inp("x", [128, 16, S])
    wq_d = inp("wq", [128, 4, 16, 128])      # m-major: [p, m, db, 128]
    wkv_d = inp("wkv", [128, 16, 256])
    wc_d = inp("wc", [128, 4, 2048])
    cos_d = inp("cos2", [128, S], f32)
    sin_d = inp("ss", [128, S], f32)
    dmask_d = inp("dmask", [128, 128])       # bf16, applied via PE matmul
    ident_d = inp("ident", [128, 128])
    out_d = nc.declare_dram_parameter("out", [S, D_MODEL], f32, isOutput=True).ap()

    EXP = mybir.ActivationFunctionType.Exp
    COPY = mybir.ActivationFunctionType.Copy

    with tile.TileContext(nc) as tc:
        with (
            tc.tile_pool(name="consts", bufs=1) as consts,
            tc.tile_pool(name="qpool", bufs=4) as qpool,
            tc.tile_pool(name="vch", bufs=2) as vchp,
            tc.tile_pool(name="tmp", bufs=2) as tmpp,
            tc.tile_pool(name="epool", bufs=8) as epool,
            tc.tile_pool(name="rpool", bufs=8) as rpool,
            tc.tile_pool(name="otcp", bufs=2) as otcp,
            tc.tile_pool(name="otTp", bufs=2) as otTp,
            tc.tile_pool(name="osb", bufs=4) as osbp,
            tc.tile_pool(name="psum_st", bufs=2, space="PSUM") as psum_st,
            tc.tile_pool(name="psum_av", bufs=4, space="PSUM") as psum_av,
        ):
            # ---- constants / weights ----
            wq_sb = consts.tile([128, 4, 16, 128], bf16, tag="wq")
            wkv_sb = consts.tile([128, 16, 256], bf16, tag="wkv")
            wc_sb = consts.tile([128, 4, 2048], bf16, tag="wc")
            c2_sb = consts.tile([128, S], f32, tag="cos2")
            ss_sb = consts.tile([128, S], f32, tag="ss")
            dmask_sb = consts.tile([128, 128], bf16, tag="dmask")
            ident_sb = consts.tile([128, 128], bf16, tag="ident")
            kTr_sb = consts.tile([128, S], bf16, tag="kTr")
            v_sb = consts.tile([128, 16, 256], bf16, tag="V")
            xT = consts.tile([128, 16, S], bf16, tag="xT")

            # input loads ordered by first use: attention goes chunk 3 -> 0,
            # so x arrives by token range (high chunks first) split across
            # the sync + vector queues; weights lead on the scalar queue and
            # the rope tables ride the gpsimd (SWDGE) queue.
            nc.scalar.dma_start(out=wq_sb[:, 0, :, :], in_=wq_d[:, 0, :, :])
            for c in (3, 2, 1, 0):
                t0 = c * 512
                nc.sync.dma_start(
                    out=xT[:, 0:8, t0:t0 + 512], in_=x_d[:, 0:8, t0:t0 + 512]
                )
                nc.gpsimd.dma_start(
                    out=xT[:, 8:16, t0:t0 + 512], in_=x_d[:, 8:16, t0:t0 + 512]
                )
                if c == 3:
                    nc.gpsimd.dma_start(out=c2_sb, in_=cos_d)
                    nc.gpsimd.dma_start(out=ss_sb, in_=sin_d)
            nc.scalar.dma_start(out=wkv_sb, in_=wkv_d)
            nc.scalar.dma_start(out=dmask_sb, in_=dmask_d)
            nc.scalar.dma_start(out=ident_sb, in_=ident_d)
            nc.scalar.dma_start(out=wq_sb[:, 1:4, :, :], in_=wq_d[:, 1:4, :, :])
            nc.gpsimd.memset(v_sb[:, :, 128:129], 1.0)

            def rope(dst, src, c):
                """dst[128,512] (bf16 SBUF) <- rotate(src[128,512] f32 PSUM).

                Row p<64 holds the even (te) element of pair p, row p+64 the
                odd (to): dst_lo = te*cos - to*sin; dst_hi = to*cos + te*sin.
                """
                cs = c2_sb[:, c * 512:(c + 1) * 512]
                sn = ss_sb[:, c * 512:(c + 1) * 512]
                t = tmpp.tile([128, 512], f32, tag="ropesin")
                t2 = tmpp.tile([128, 512], f32, tag="ropecos")
                nc.vector.tensor_mul(t[0:64, :], src[64:128, :], sn[0:64, :])
                nc.vector.tensor_mul(t[64:128, :], src[0:64, :], sn[64:128, :])
                nc.vector.tensor_mul(t2, src, cs)
                nc.vector.tensor_add(dst, t2, t)

            qTrs = {}
            proj_pool_cm = tc.tile_pool(name="psum_proj", bufs=2, space="PSUM")
            psum = proj_pool_cm.__enter__()

            def gen_qproj(c, ms=(0, 1, 2, 3)):
                """Generator: emits Q projection for chunk c, yielding every
                2 accumulation steps (~2x512-row matmuls) for interleaving."""
                tq0 = c * 512
                if c in qTrs:
                    qTr = qTrs[c]
                else:
                    qTr = qpool.tile([128, 4, 512], bf16, tag="qTr", name=f"qTr{c}")
                    qTrs[c] = qTr
                for m in ms:
                    pq = psum.tile([128, 512], f32, tag="mm512")
                    for db in range(16):
                        nc.tensor.matmul(
                            pq,
                            lhsT=wq_sb[:, m, db, :],
                            rhs=xT[:, db, tq0:tq0 + 512],
                            start=(db == 0),
                            stop=(db == 15),
                        )
                        if db % 2 == 1:
                            yield
                    rope(qTr[:, m, :], pq, c)

            def gen_kvproj(c):
                tq0 = c * 512
                pk = psum.tile([128, 512], f32, tag="mm512")
                for db in range(16):
                    nc.tensor.matmul(
                        pk,
                        lhsT=wkv_sb[:, db, 0:128],
                        rhs=xT[:, db, tq0:tq0 + 512],
                        start=(db == 0),
                        stop=(db == 15),
                    )
                    if db % 2 == 1:
                        yield
                rope(kTr_sb[:, tq0:tq0 + 512], pk, c)
                pv = psum.tile([128, 512], f32, tag="mm512")
                for db in range(16):
                    nc.tensor.matmul(
                        pv,
                        lhsT=wkv_sb[:, db, 128:256],
                        rhs=xT[:, db, tq0:tq0 + 512],
                        start=(db == 0),
                        stop=(db == 15),
                    )
                    if db % 2 == 1:
                        yield
                vch = vchp.tile([128, 512], bf16, tag="vch")
                nc.scalar.copy(out=vch, in_=pv)
                # per-block 2D DMA transposes: strided 3D-out DMAT corrupts
                # data in the XBAR path, so keep each out contiguous
                for rr in range(4):
                    nc.sync.dma_start_transpose(
                        out=v_sb[:, 4 * c + rr, 0:128],
                        in_=vch[:, rr * 128:(rr + 1) * 128],
                    )

            def run_gen(g):
                for _ in g:
                    pass

            # ---- projection prologue: everything attention chunk 3 needs ----
            run_gen(gen_qproj(3, ms=(0,)))
            run_gen(gen_kvproj(3))
            run_gen(gen_qproj(3, ms=(1, 2, 3)))
            run_gen(gen_kvproj(2))
            run_gen(gen_kvproj(1))
            run_gen(gen_kvproj(0))
            nc.scalar.dma_start(out=wc_sb, in_=wc_d)

            # ---- fillers: remaining projections, then out-projections ----
            out_pool_cm = [None]
            psum_o = [None]

            def gen_outproj(tq0, otT):
                """Generator: out-projection for chunk at tq0 from otT
                [128d, 4h, 512t]; yields after each psum tile (4 matmuls)."""
                for tb in range(4):
                    row = tq0 + tb * 128
                    for oc in range(4):
                        po = psum_o[0].tile([128, 512], f32, tag="out")
                        for h in range(4):
                            nc.tensor.matmul(
                                po,
                                lhsT=otT[:, h, tb, :],
                                rhs=wc_sb[:, h, oc * 512:(oc + 1) * 512],
                                start=(h == 0),
                                stop=(h == 3),
                            )
                        osb = osbp.tile([128, 512], f32, tag="osb")
                        if oc % 2 == 0:
                            nc.vector.tensor_copy(out=osb, in_=po)
                        else:
                            nc.scalar.copy(out=osb, in_=po)
                        nc.sync.dma_start(
                            out=out_d[row:row + 128, oc * 512:(oc + 1) * 512],
                            in_=osb,
                        )
                        yield

            proj_gens = [gen_qproj(2), gen_qproj(1), gen_qproj(0)]
            out_gens = []

            def pull_filler(n=1):
                """Emit up to n filler steps: projections first, then
                out-projections (swapping PSUM pools at the boundary)."""
                while n > 0:
                    if proj_gens:
                        try:
                            next(proj_gens[0])
                            n -= 1
                        except StopIteration:
                            proj_gens.pop(0)
                            if not proj_gens:
                                # all projections done: swap proj psum pool
                                # for the out-projection pool (2 banks each)
                                proj_pool_cm.__exit__(None, None, None)
                                cm = tc.tile_pool(
                                    name="psum_o", bufs=2, space="PSUM"
                                )
                                out_pool_cm[0] = cm
                                psum_o[0] = cm.__enter__()
                    elif out_gens:
                        try:
                            next(out_gens[0])
                            n -= 1
                        except StopIteration:
                            out_gens.pop(0)
                    else:
                        return

            # ---- attention chunks, biggest first ----
            pending_outproj = []
            for c in (3, 2, 1, 0):
                tq0 = c * 512
                qTr = qTrs[c]
                nkb = 4 * c + 4
                otT_c = otTp.tile([128, 4, 4, 128], bf16, tag="otT", name=f"otT{c}")
                for h in range(4):
                    if h == 1 and pending_outproj:
                        # release the previous chunk's out-projection only
                        # after one head of slack so its transposes land
                        out_gens.append(pending_outproj.pop(0))
                    otc = otcp.tile([128, 4, 128], bf16, tag="otc")
                    pav = [
                        psum_av.tile([128, 129], f32, tag="pav", name=f"pav{tb}")
                        for tb in range(4)
                    ]

                    def st_mm(kb):
                        """Score matmul for one key block (cols col0:512).

                        Diagonal blocks get the causal mask accumulated as a
                        second PE matmul (identity.T @ dmask == dmask), which
                        keeps the scores->exp chain entirely on the PE.
                        """
                        rr = kb - 4 * c
                        col0 = 0 if rr < 0 else 128 * rr
                        pst = psum_st.tile([128, 512], f32, tag="st")
                        nc.tensor.matmul(
                            pst[:, col0:512],
                            lhsT=kTr_sb[:, kb * 128:(kb + 1) * 128],
                            rhs=qTr[:, h, col0:512],
                            start=True,
                            stop=(rr < 0),
                            skip_group_check=True,
                        )
                        if rr >= 0:
                            nc.tensor.matmul(
                                pst[:, col0:col0 + 128],
                                lhsT=ident_sb,
                                rhs=dmask_sb,
                                start=False,
                                stop=True,
                                skip_group_check=True,
                            )
                        return pst, col0

                    # software-pipelined: emit S^T(kb+1) ahead of the
                    # exp-dependent AV matmuls of kb so the PE stream never
                    # head-blocks on the activation engine
                    pending = st_mm(0)
                    for kb in range(nkb):
                        pst, col0 = pending
                        if kb + 1 < nkb:
                            pending = st_mm(kb + 1)
                        e = epool.tile([128, 512], bf16, tag="E")
                        nc.scalar.activation(
                            out=e[:, col0:512], in_=pst[:, col0:512], func=EXP
                        )
                        rr = kb - 4 * c
                        tb_lo = rr if rr > 0 else 0
                        for tb in range(tb_lo, 4):
                            nc.tensor.matmul(
                                pav[tb],
                                lhsT=e[:, tb * 128:(tb + 1) * 128],
                                rhs=v_sb[:, kb, 0:129],
                                start=(kb == 0),
                                stop=(kb == 4 * c + tb),
                            )
                        pull_filler(1)

                    # normalize: per-token (per-partition) reciprocal of the
                    # accumulated 129th column, then one blocked
                    # DMA-transpose [t,d] -> [d,t] for the out-proj
                    for tb in range(4):
                        r = rpool.tile([128, 1], f32, tag="r")
                        nc.vector.reciprocal_approx_fast(
                            out=r, in_=pav[tb][:, 128:129]
                        )
                        if tb % 2 == 0:
                            nc.vector.tensor_scalar_mul(
                                otc[:, tb, :], pav[tb][:, 0:128], r
                            )
                        else:
                            nc.scalar.activation(
                                out=otc[:, tb, :], in_=pav[tb][:, 0:128],
                                func=COPY, scale=r,
                            )
                    nc.sync.dma_start_transpose(
                        out=otT_c[:, h, :, :], in_=otc[:, :, :]
                    )
                    pull_filler(2)

                # queue this chunk's out-projection as filler work (released
                # one head into the next chunk's attention)
                pending_outproj.append(gen_outproj(tq0, otT_c))
            while pending_outproj:
                out_gens.append(pending_outproj.pop(0))

            # drain remaining fillers (last chunk's outproj + leftovers)
            pull_filler(10**9)
            if out_pool_cm[0] is not None:
                out_pool_cm[0].__exit__(None, None, None)

    nc.compile()
    return nc


def _host_prep(x, Wq, Wkv, Wc):
    """Shard + relayout the full inputs into the 8 per-core input dicts."""
    import ml_dtypes

    bf = ml_dtypes.bfloat16
    dk, H, KV = DK, N_HEADS, N_KV_HEADS
    x = np.asarray(x, np.float32)
    Wq = np.asarray(Wq, np.float32)
    Wkv = np.asarray(Wkv, np.float32)
    Wc = np.asarray(Wc, np.float32)

    p = np.concatenate([np.arange(0, dk, 2), np.arange(1, dk, 2)])
    perm_q = np.concatenate([h * dk + p for h in range(H)])
    Wq_p = (Wq / math.sqrt(dk))[perm_q]
    perm_k = np.concatenate([g * dk + p for g in range(KV)])
    Wk_p = Wkv[:KV * dk][perm_k]
    Wv = Wkv[KV * dk:]

    pairs = np.arange(dk // 2, dtype=np.float64)
    freqs = 1.0 / (ROPE_THETA ** (2.0 * pairs / dk))
    ang = np.arange(S, dtype=np.float64)[:, None] * freqs[None, :]
    cos_t = np.cos(ang).astype(np.float32).T  # [64, S]
    sin_t = np.sin(ang).astype(np.float32).T
    c2 = np.ascontiguousarray(np.concatenate([cos_t, cos_t], 0))   # [128, S]
    ss = np.ascontiguousarray(np.concatenate([-sin_t, sin_t], 0))  # [128, S]

    jj = np.arange(128)[None, :]
    pp = np.arange(128)[:, None]
    dmask = np.where(pp <= jj, 0.0, NEG).astype(bf)
    ident = np.eye(128, dtype=bf)

    maps = []
    for core in range(NCORES):
        b, g = core // 4, core % 4
        # [p, m, db, 128]: feature-partition p, head m, contraction block db
        wq_l = np.ascontiguousarray(
            Wq_p[512 * g:512 * g + 512].T
            .reshape(16, 128, 4, 128).transpose(1, 2, 0, 3)
        ).astype(bf)
        wkv_sl = np.concatenate(
            [Wk_p[g * dk:(g + 1) * dk], Wv[g * dk:(g + 1) * dk]], 0
        ).T  # [2048, 256]
        wkv_l = np.ascontiguousarray(
            wkv_sl.reshape(16, 128, 256).transpose(1, 0, 2)
        ).astype(bf)
        wc_l = np.ascontiguousarray(
            Wc[:, 512 * g:512 * g + 512].T.reshape(4, 128, 2048).transpose(1, 0, 2)
        ).astype(bf)
        xt_l = np.ascontiguousarray(
            x[b].T.reshape(16, 128, S).transpose(1, 0, 2)
        ).astype(bf)
        maps.append(dict(
            x=xt_l, wq=wq_l, wkv=wkv_l, wc=wc_l,
            cos2=c2, ss=ss, dmask=dmask, ident=ident,
        ))
    return maps


def kernel(x, Wq, Wkv, Wc):
    global _COMPILED, _LAST_RESULT
    from concourse.bass_utils import run_bass_kernel_spmd

    if _COMPILED is None:
        _COMPILED = _build()
    in_maps = _host_prep(x, Wq, Wkv, Wc)
    res = run_bass_kernel_spmd(
        _COMPILED, in_maps, core_ids=list(range(NCORES)), trace=_TRACE
    )
    _LAST_RESULT = res
    outs = [res.results[i]["out"] for i in range(NCORES)]
    full = np.stack(
        [outs[0] + outs[1] + outs[2] + outs[3],
         outs[4] + outs[5] + outs[6] + outs[7]], 0
    ).astype(np.float32)
    return full


# revision 47
# speedup vs baseline: 1.0070x; 1.0046x over previous
"""Causal self-attention (GQA + RoPE) Trainium2 Bass kernel, 8 NeuronCores.

Sharding: 2-way data parallel over batch x 4-way tensor parallel over heads.
Core c handles batch c//4 and query heads [4*(c%4), 4*(c%4)+4) plus the one
KV head g = c%4 that serves them (n_kv_heads=4 -> no KV replication).
Each core computes a partial [S, D] output (its heads' slice of the out
projection); the host sums the 4 partials per batch.

Device layouts are transposed ("feature-major"): x is loaded pre-transposed;
projections produce qT/kT [dim, tokens]; attention scores are computed as
S^T = kT.T @ qT.  The P@V contraction is TOKEN-major: per 128-token block,
matmul(out[128t,129], lhsT=e_block[128k,128t], rhs=[v|ones][128k,129]) - the
129th column accumulates the softmax denominator for free, so there are no
separate row-sum matmuls and normalization is a per-partition (per-token)
scale.  The normalized [token, dim] tiles are DMA-transposed back to
[dim, token] for the output projection.  RoPE is handled by de-interleaving
the q/k weight rows on the host so rotation pairs become (p, p+64) partition
pairs.  TensorEngine-facing tensors are bf16 (fp32 PSUM accumulation).

The PE stream is kept dense by interleaving projection / out-projection
matmul groups as fillers inside the (Scalar-bound) attention kb loop.
"""

import sys

if "/opt/trn_rl_repo" not in sys.path:
    sys.path.insert(0, "/opt/trn_rl_repo")

import math

import numpy as np

D_MODEL = 2048
N_HEADS = 16
N_KV_HEADS = 4
ROPE_THETA = 10000.0
B, S = 2, 2048
DK = D_MODEL // N_HEADS          # 128
NCORES = 8
NEG = -1e30

_COMPILED = None
_TRACE = False                   # test.py flips this for profiling runs
_LAST_RESULT = None              # BassKernelResults of the last run


def _build():
    import concourse.bacc as bacc
    import concourse.tile as tile
    from concourse import mybir

    f32 = mybir.dt.float32
    bf16 = mybir.dt.bfloat16

    nc = bacc.Bacc("TRN2", debug=False, target_bir_lowering=False)

    def inp(name, shape, dt=bf16):
        return nc.declare_dram_parameter(name, list(shape), dt, isOutput=False).ap()

    x_d = inp("x", [128, 16, S])
    wq_d = inp("wq", [128, 4, 16, 128])      # m-major: [p, m, db, 128]
    wkv_d = inp("wkv", [128, 16, 256])
    wc_d = inp("wc", [128, 4, 2048])
    cos_d = inp("cos2", [128, S], f32)
    sin_d = inp("ss", [128, S], f32)
    dmask_d = inp("dmask", [128, 128])       # bf16, applied via PE matmul
    ident_d = inp("ident", [128, 128])
    out_d = nc.declare_dram_parameter("out", [S, D_MODEL], f32, isOutput=True).ap()

    EXP = mybir.ActivationFunctionType.Exp
    COPY = mybir.ActivationFunctionType.Copy

    with tile.TileContext(nc) as tc:
        with (
            tc.tile_pool(name="consts", bufs=1) as consts,
            tc.tile_pool(name="qpool", bufs=4) as qpool,
            tc.tile_pool(name="vch", bufs=2) as vchp,
            tc.tile_pool(name="tmp", bufs=2) as tmpp,
            tc.tile_pool(name="epool", bufs=8) as epool,
            tc.tile_pool(name="rpool", bufs=8) as rpool,
            tc.tile_pool(name="otcp", bufs=2) as otcp,
            tc.tile_pool(name="otTp", bufs=2) as otTp,
            tc.tile_pool(name="osb", bufs=4) as osbp,
            tc.tile_pool(name="psum_st", bufs=2, space="PSUM") as psum_st,
            tc.tile_pool(name="psum_av", bufs=4, space="PSUM") as psum_av,
        ):
            # ---- constants / weights ----
            wq_sb = consts.tile([128, 4, 16, 128], bf16, tag="wq")
            wkv_sb = consts.tile([128, 16, 256], bf16, tag="wkv")
            wc_sb = consts.tile([128, 4, 2048], bf16, tag="wc")
            c2_sb = consts.tile([128, S], f32, tag="cos2")
            ss_sb = consts.tile([128, S], f32, tag="ss")
            dmask_sb = consts.tile([128, 128], bf16, tag="dmask")
            ident_sb = consts.tile([128, 128], bf16, tag="ident")
            kTr_sb = consts.tile([128, S], bf16, tag="kTr")
            v_sb = consts.tile([128, 16, 256], bf16, tag="V")
            xT = consts.tile([128, 16, S], bf16, tag="xT")

            # input loads ordered by first use: attention goes chunk 3 -> 0,
            # so x arrives by token range (high chunks first) split across
            # the sync + vector queues; weights lead on the scalar queue and
            # the rope tables ride the gpsimd (SWDGE) queue.
            nc.scalar.dma_start(out=wq_sb[:, 0, :, :], in_=wq_d[:, 0, :, :])
            for c in (3, 2, 1, 0):
                t0 = c * 512
                nc.sync.dma_start(
                    out=xT[:, 0:8, t0:t0 + 512], in_=x_d[:, 0:8, t0:t0 + 512]
                )
                nc.gpsimd.dma_start(
                    out=xT[:, 8:16, t0:t0 + 512], in_=x_d[:, 8:16, t0:t0 + 512]
                )
                if c == 3:
                    nc.gpsimd.dma_start(out=c2_sb, in_=cos_d)
                    nc.gpsimd.dma_start(out=ss_sb, in_=sin_d)
            nc.scalar.dma_start(out=wkv_sb, in_=wkv_d)
            nc.scalar.dma_start(out=dmask_sb, in_=dmask_d)
            nc.scalar.dma_start(out=ident_sb, in_=ident_d)
            nc.scalar.dma_start(out=wq_sb[:, 1:4, :, :], in_=wq_d[:, 1:4, :, :])
            nc.gpsimd.memset(v_sb[:, :, 128:129], 1.0)

            def rope(dst, src, c):
                """dst[128,512] (bf16 SBUF) <- rotate(src[128,512] f32 PSUM).

                Row p<64 holds the even (te) element of pair p, row p+64 the
                odd (to): dst_lo = te*cos - to*sin; dst_hi = to*cos + te*sin.
                """
                cs = c2_sb[:, c * 512:(c + 1) * 512]
                sn = ss_sb[:, c * 512:(c + 1) * 512]
                t = tmpp.tile([128, 512], f32, tag="ropesin")
                t2 = tmpp.tile([128, 512], f32, tag="ropecos")
                nc.vector.tensor_mul(t[0:64, :], src[64:128, :], sn[0:64, :])
                nc.vector.tensor_mul(t[64:128, :], src[0:64, :], sn[64:128, :])
                nc.vector.tensor_mul(t2, src, cs)
                nc.vector.tensor_add(dst, t2, t)

            qTrs = {}
            proj_pool_cm = tc.tile_pool(name="psum_proj", bufs=2, space="PSUM")
            psum = proj_pool_cm.__enter__()

            def gen_qproj(c, ms=(0, 1, 2, 3)):
                """Generator: emits Q projection for chunk c, yielding every
                2 accumulation steps (~2x512-row matmuls) for interleaving."""
                tq0 = c * 512
                if c in qTrs:
                    qTr = qTrs[c]
                else:
                    qTr = qpool.tile([128, 4, 512], bf16, tag="qTr", name=f"qTr{c}")
                    qTrs[c] = qTr
                for m in ms:
                    pq = psum.tile([128, 512], f32, tag="mm512")
                    for db in range(16):
                        nc.tensor.matmul(
                            pq,
                            lhsT=wq_sb[:, m, db, :],
                            rhs=xT[:, db, tq0:tq0 + 512],
                            start=(db == 0),
                            stop=(db == 15),
                        )
                        if db % 2 == 1:
                            yield
                    rope(qTr[:, m, :], pq, c)

            def gen_kvproj(c):
                tq0 = c * 512
                pk = psum.tile([128, 512], f32, tag="mm512")
                for db in range(16):
                    nc.tensor.matmul(
                        pk,
                        lhsT=wkv_sb[:, db, 0:128],
                        rhs=xT[:, db, tq0:tq0 + 512],
                        start=(db == 0),
                        stop=(db == 15),
                    )
                    if db % 2 == 1:
                        yield
                rope(kTr_sb[:, tq0:tq0 + 512], pk, c)
                pv = psum.tile([128, 512], f32, tag="mm512")
                for db in range(16):
                    nc.tensor.matmul(
                        pv,
                        lhsT=wkv_sb[:, db, 128:256],
                        rhs=xT[:, db, tq0:tq0 + 512],
                        start=(db == 0),
                        stop=(db == 15),
                    )
                    if db % 2 == 1:
                        yield
                vch = vchp.tile([128, 512], bf16, tag="vch")
                nc.scalar.copy(out=vch, in_=pv)
                # per-block 2D DMA transposes: strided 3D-out DMAT corrupts
                # data in the XBAR path, so keep each out contiguous
                for rr in range(4):
                    nc.sync.dma_start_transpose(
                        out=v_sb[:, 4 * c + rr, 0:128],
                        in_=vch[:, rr * 128:(rr + 1) * 128],
                    )

            def run_gen(g):
                for _ in g:
                    pass

            # ---- projection prologue: everything attention chunk 3 needs ----
            run_gen(gen_qproj(3, ms=(0,)))
            run_gen(gen_kvproj(3))
            run_gen(gen_qproj(3, ms=(1, 2, 3)))
            run_gen(gen_kvproj(2))
            run_gen(gen_kvproj(1))
            run_gen(gen_kvproj(0))
            nc.scalar.dma_start(out=wc_sb, in_=wc_d)

            # ---- fillers: remaining projections, then out-projections ----
            out_pool_cm = [None]
            psum_o = [None]

            def gen_outproj(tq0, otT):
                """Generator: out-projection for chunk at tq0 from otT
                [128d, 4h, 512t]; yields after each psum tile (4 matmuls)."""
                for tb in range(4):
                    row = tq0 + tb * 128
                    for oc in range(4):
                        po = psum_o[0].tile([128, 512], f32, tag="out")
                        for h in range(4):
                            nc.tensor.matmul(
                                po,
                                lhsT=otT[:, h, tb, :],
                                rhs=wc_sb[:, h, oc * 512:(oc + 1) * 512],
                                start=(h == 0),
                                stop=(h == 3),
                            )
                        osb = osbp.tile([128, 512], f32, tag="osb")
                        nc.vector.tensor_copy(out=osb, in_=po)
                        nc.sync.dma_start(
                            out=out_d[row:row + 128, oc * 512:(oc + 1) * 512],
                            in_=osb,
                        )
                        yield

            proj_gens = [gen_qproj(2), gen_qproj(1), gen_qproj(0)]
            out_gens = []

            def pull_filler(n=1):
                """Emit up to n filler steps: projections first, then
                out-projections (swapping PSUM pools at the boundary)."""
                while n > 0:
                    if proj_gens:
                        try:
                            next(proj_gens[0])
                            n -= 1
                        except StopIteration:
                            proj_gens.pop(0)
                            if not proj_gens:
                                # all projections done: swap proj psum pool
                                # for the out-projection pool (2 banks each)
                                proj_pool_cm.__exit__(None, None, None)
                                cm = tc.tile_pool(
                                    name="psum_o", bufs=2, space="PSUM"
                                )
                                out_pool_cm[0] = cm
                                psum_o[0] = cm.__enter__()
                    elif out_gens:
                        try:
                            next(out_gens[0])
                            n -= 1
                        except StopIteration:
                            out_gens.pop(0)
                    else:
                        return

            # ---- attention chunks, biggest first ----
            pending_outproj = []
            for c in (3, 2, 1, 0):
                tq0 = c * 512
                qTr = qTrs[c]
                nkb = 4 * c + 4
                otT_c = otTp.tile([128, 4, 4, 128], bf16, tag="otT", name=f"otT{c}")
                for h in range(4):
                    if h == 1 and pending_outproj:
                        # release the previous chunk's out-projection only
                        # after one head of slack so its transposes land
                        out_gens.append(pending_outproj.pop(0))
                    otc = otcp.tile([128, 4, 128], bf16, tag="otc")
                    pav = [
                        psum_av.tile([128, 129], f32, tag="pav", name=f"pav{tb}")
                        for tb in range(4)
                    ]

                    def st_mm(kb):
                        """Score matmul for one key block (cols col0:512).

                        Diagonal blocks get the causal mask accumulated as a
                        second PE matmul (identity.T @ dmask == dmask), which
                        keeps the scores->exp chain entirely on the PE.
                        """
                        rr = kb - 4 * c
                        col0 = 0 if rr < 0 else 128 * rr
                        pst = psum_st.tile([128, 512], f32, tag="st")
                        nc.tensor.matmul(
                            pst[:, col0:512],
                            lhsT=kTr_sb[:, kb * 128:(kb + 1) * 128],
                            rhs=qTr[:, h, col0:512],
                            start=True,
                            stop=(rr < 0),
                            skip_group_check=True,
                        )
                        if rr >= 0:
                            nc.tensor.matmul(
                                pst[:, col0:col0 + 128],
                                lhsT=ident_sb,
                                rhs=dmask_sb,
                                start=False,
                                stop=True,
                                skip_group_check=True,
                            )
                        return pst, col0

                    # software-pipelined: emit S^T(kb+1) ahead of the
                    # exp-dependent AV matmuls of kb so the PE stream never
                    # head-blocks on the activation engine
                    pending = st_mm(0)
                    for kb in range(nkb):
                        pst, col0 = pending
                        if kb + 1 < nkb:
                            pending = st_mm(kb + 1)
                        e = epool.tile([128, 512], bf16, tag="E")
                        nc.scalar.activation(
                            out=e[:, col0:512], in_=pst[:, col0:512], func=EXP
                        )
                        rr = kb - 4 * c
                        tb_lo = rr if rr > 0 else 0
                        for tb in range(tb_lo, 4):
                            nc.tensor.matmul(
                                pav[tb],
                                lhsT=e[:, tb * 128:(tb + 1) * 128],
                                rhs=v_sb[:, kb, 0:129],
                                start=(kb == 0),
                                stop=(kb == 4 * c + tb),
                            )
                        pull_filler(1)

                    # normalize: per-token (per-partition) reciprocal of the
                    # accumulated 129th column, then one blocked
                    # DMA-transpose [t,d] -> [d,t] for the out-proj
                    for tb in range(4):
                        r = rpool.tile([128, 1], f32, tag="r")
                        nc.vector.reciprocal_approx_fast(
                            out=r, in_=pav[tb][:, 128:129]
                        )
                        if tb % 2 == 0:
                            nc.vector.tensor_scalar_mul(
                                otc[:, tb, :], pav[tb][:, 0:128], r
                            )
                        else:
                            nc.scalar.activation(
                                out=otc[:, tb, :], in_=pav[tb][:, 0:128],
                                func=COPY, scale=r,
                            )
                    nc.sync.dma_start_transpose(
                        out=otT_c[:, h, :, :], in_=otc[:, :, :]
                    )
                    pull_filler(2)

                # queue this chunk's out-projection as filler work (released
                # one head into the next chunk's attention)
                pending_outproj.append(gen_outproj(tq0, otT_c))
            while pending_outproj:
                out_gens.append(pending_outproj.pop(0))

            # drain remaining fillers (last chunk's outproj + leftovers)
            pull_filler(10**9)
            if out_pool_cm[0] is not None:
                out_pool_cm[0].__exit__(None, None, None)

    nc.compile()
    return nc


def _host_prep(x, Wq, Wkv, Wc):
    """Shard + relayout the full inputs into the 8 per-core input dicts."""
    import ml_dtypes

    bf = ml_dtypes.bfloat16
    dk, H, KV = DK, N_HEADS, N_KV_HEADS
    x = np.asarray(x, np.float32)
    Wq = np.asarray(Wq, np.float32)
    Wkv = np.asarray(Wkv, np.float32)
    Wc = np.asarray(Wc, np.float32)

    p = np.concatenate([np.arange(0, dk, 2), np.arange(1, dk, 2)])
    perm_q = np.concatenate([h * dk + p for h in range(H)])
    Wq_p = (Wq / math.sqrt(dk))[perm_q]
    perm_k = np.concatenate([g * dk + p for g in range(KV)])
    Wk_p = Wkv[:KV * dk][perm_k]
    Wv = Wkv[KV * dk:]

    pairs = np.arange(dk // 2, dtype=np.float64)
    freqs = 1.0 / (ROPE_THETA ** (2.0 * pairs / dk))
    ang = np.arange(S, dtype=np.float64)[:, None] * freqs[None, :]
    cos_t = np.cos(ang).astype(np.float32).T  # [64, S]
    sin_t = np.sin(ang).astype(np.float32).T
    c2 = np.ascontiguousarray(np.concatenate([cos_t, cos_t], 0))   # [128, S]
    ss = np.ascontiguousarray(np.concatenate([-sin_t, sin_t], 0))  # [128, S]

    jj = np.arange(128)[None, :]
    pp = np.arange(128)[:, None]
    dmask = np.where(pp <= jj, 0.0, NEG).astype(bf)
    ident = np.eye(128, dtype=bf)

    maps = []
    for core in range(NCORES):
        b, g = core // 4, core % 4
        # [p, m, db, 128]: feature-partition p, head m, contraction block db
        wq_l = np.ascontiguousarray(
            Wq_p[512 * g:512 * g + 512].T
            .reshape(16, 128, 4, 128).transpose(1, 2, 0, 3)
        ).astype(bf)
        wkv_sl = np.concatenate(
            [Wk_p[g * dk:(g + 1) * dk], Wv[g * dk:(g + 1) * dk]], 0
        ).T  # [2048, 256]
        wkv_l = np.ascontiguousarray(
            wkv_sl.reshape(16, 128, 256).transpose(1, 0, 2)
        ).astype(bf)
        wc_l = np.ascontiguousarray(
            Wc[:, 512 * g:512 * g + 512].T.reshape(4, 128, 2048).transpose(1, 0, 2)
        ).astype(bf)
        xt_l = np.ascontiguousarray(
            x[b].T.reshape(16, 128, S).transpose(1, 0, 2)
        ).astype(bf)
        maps.append(dict(
            x=xt_l, wq=wq_l, wkv=wkv_l, wc=wc_l,
            cos2=c2, ss=ss, dmask=dmask, ident=ident,
        ))
    return maps


def kernel(x, Wq, Wkv, Wc):
    global _COMPILED, _LAST_RESULT
    from concourse.bass_utils import run_bass_kernel_spmd

    if _COMPILED is None:
        _COMPILED = _build()
    in_maps = _host_prep(x, Wq, Wkv, Wc)
    res = run_bass_kernel_spmd(
        _COMPILED, in_maps, core_ids=list(range(NCORES)), trace=_TRACE
    )
    _LAST_RESULT = res
    outs = [res.results[i]["out"] for i in range(NCORES)]
    full = np.stack(
        [outs[0] + outs[1] + outs[2] + outs[3],
         outs[4] + outs[5] + outs[6] + outs[7]], 0
    ).astype(np.float32)
    return full


# revision 48
# speedup vs baseline: 1.0232x; 1.0161x over previous
"""Causal self-attention (GQA + RoPE) Trainium2 Bass kernel, 8 NeuronCores.

Sharding: 2-way data parallel over batch x 4-way tensor parallel over heads.
Core c handles batch c//4 and query heads [4*(c%4), 4*(c%4)+4) plus the one
KV head g = c%4 that serves them (n_kv_heads=4 -> no KV replication).
Each core computes a partial [S, D] output (its heads' slice of the out
projection); the host sums the 4 partials per batch.

Device layouts are transposed ("feature-major"): x is loaded pre-transposed;
projections produce qT/kT [dim, tokens]; attention scores are computed as
S^T = kT.T @ qT.  The P@V contraction is TOKEN-major: per 128-token block,
matmul(out[128t,129], lhsT=e_block[128k,128t], rhs=[v|ones][128k,129]) - the
129th column accumulates the softmax denominator for free, so there are no
separate row-sum matmuls and normalization is a per-partition (per-token)
scale.  The normalized [token, dim] tiles are DMA-transposed back to
[dim, token] for the output projection.  RoPE is handled by de-interleaving
the q/k weight rows on the host so rotation pairs become (p, p+64) partition
pairs.  TensorEngine-facing tensors are bf16 (fp32 PSUM accumulation).

The PE stream is kept dense by interleaving projection / out-projection
matmul groups as fillers inside the (Scalar-bound) attention kb loop.
"""

import sys

if "/opt/trn_rl_repo" not in sys.path:
    sys.path.insert(0, "/opt/trn_rl_repo")

import math

import numpy as np

D_MODEL = 2048
N_HEADS = 16
N_KV_HEADS = 4
ROPE_THETA = 10000.0
B, S = 2, 2048
DK = D_MODEL // N_HEADS          # 128
NCORES = 8
NEG = -1e30

_COMPILED = None
_TRACE = False                   # test.py flips this for profiling runs
_LAST_RESULT = None              # BassKernelResults of the last run


def _build():
    import concourse.bacc as bacc
    import concourse.tile as tile
    from concourse import mybir

    f32 = mybir.dt.float32
    bf16 = mybir.dt.bfloat16

    nc = bacc.Bacc("TRN2", debug=False, target_bir_lowering=False)

    def inp(name, shape, dt=bf16):
        return nc.declare_dram_parameter(name, list(shape), dt, isOutput=False).ap()

    x_d = inp("x", [128, 16, S])
    wq_d = inp("wq", [128, 4, 16, 128])      # m-major: [p, m, db, 128]
    wkv_d = inp("wkv", [128, 16, 256])
    wc_d = inp("wc", [128, 4, 2048])
    cos_d = inp("cos2", [128, S], f32)
    sin_d = inp("ss", [128, S], f32)
    dmask_d = inp("dmask", [128, 128])       # bf16, applied via PE matmul
    ident_d = inp("ident", [128, 128])
    out_d = nc.declare_dram_parameter("out", [S, D_MODEL], f32, isOutput=True).ap()

    EXP = mybir.ActivationFunctionType.Exp
    COPY = mybir.ActivationFunctionType.Copy

    with tile.TileContext(nc) as tc:
        with (
            tc.tile_pool(name="consts", bufs=1) as consts,
            tc.tile_pool(name="qpool", bufs=4) as qpool,
            tc.tile_pool(name="vch", bufs=2) as vchp,
            tc.tile_pool(name="tmp", bufs=2) as tmpp,
            tc.tile_pool(name="epool", bufs=12) as epool,
            tc.tile_pool(name="rpool", bufs=8) as rpool,
            tc.tile_pool(name="otcp", bufs=4) as otcp,
            tc.tile_pool(name="otTp", bufs=2) as otTp,
            tc.tile_pool(name="osb", bufs=6) as osbp,
            tc.tile_pool(name="psum_st", bufs=2, space="PSUM") as psum_st,
            tc.tile_pool(name="psum_av", bufs=4, space="PSUM") as psum_av,
        ):
            # ---- constants / weights ----
            wq_sb = consts.tile([128, 4, 16, 128], bf16, tag="wq")
            wkv_sb = consts.tile([128, 16, 256], bf16, tag="wkv")
            wc_sb = consts.tile([128, 4, 2048], bf16, tag="wc")
            c2_sb = consts.tile([128, S], f32, tag="cos2")
            ss_sb = consts.tile([128, S], f32, tag="ss")
            dmask_sb = consts.tile([128, 128], bf16, tag="dmask")
            ident_sb = consts.tile([128, 128], bf16, tag="ident")
            kTr_sb = consts.tile([128, S], bf16, tag="kTr")
            v_sb = consts.tile([128, 16, 256], bf16, tag="V")
            xT = consts.tile([128, 16, S], bf16, tag="xT")

            # input loads ordered by first use: attention goes chunk 3 -> 0,
            # so x arrives by token range (high chunks first) split across
            # the sync + vector queues; weights lead on the scalar queue and
            # the rope tables ride the gpsimd (SWDGE) queue.
            nc.scalar.dma_start(out=wq_sb[:, 0, :, :], in_=wq_d[:, 0, :, :])
            for c in (3, 2, 1, 0):
                t0 = c * 512
                nc.sync.dma_start(
                    out=xT[:, 0:8, t0:t0 + 512], in_=x_d[:, 0:8, t0:t0 + 512]
                )
                nc.gpsimd.dma_start(
                    out=xT[:, 8:16, t0:t0 + 512], in_=x_d[:, 8:16, t0:t0 + 512]
                )
                if c == 3:
                    nc.gpsimd.dma_start(out=c2_sb, in_=cos_d)
                    nc.gpsimd.dma_start(out=ss_sb, in_=sin_d)
            nc.scalar.dma_start(out=wkv_sb, in_=wkv_d)
            nc.scalar.dma_start(out=dmask_sb, in_=dmask_d)
            nc.scalar.dma_start(out=ident_sb, in_=ident_d)
            nc.scalar.dma_start(out=wq_sb[:, 1:4, :, :], in_=wq_d[:, 1:4, :, :])
            nc.gpsimd.memset(v_sb[:, :, 128:129], 1.0)

            def rope(dst, src, c):
                """dst[128,512] (bf16 SBUF) <- rotate(src[128,512] f32 PSUM).

                Row p<64 holds the even (te) element of pair p, row p+64 the
                odd (to): dst_lo = te*cos - to*sin; dst_hi = to*cos + te*sin.
                """
                cs = c2_sb[:, c * 512:(c + 1) * 512]
                sn = ss_sb[:, c * 512:(c + 1) * 512]
                t = tmpp.tile([128, 512], f32, tag="ropesin")
                t2 = tmpp.tile([128, 512], f32, tag="ropecos")
                nc.vector.tensor_mul(t[0:64, :], src[64:128, :], sn[0:64, :])
                nc.vector.tensor_mul(t[64:128, :], src[0:64, :], sn[64:128, :])
                nc.vector.tensor_mul(t2, src, cs)
                nc.vector.tensor_add(dst, t2, t)

            qTrs = {}
            proj_pool_cm = tc.tile_pool(name="psum_proj", bufs=2, space="PSUM")
            psum = proj_pool_cm.__enter__()

            def gen_qproj(c, ms=(0, 1, 2, 3)):
                """Generator: emits Q projection for chunk c, yielding every
                2 accumulation steps (~2x512-row matmuls) for interleaving."""
                tq0 = c * 512
                if c in qTrs:
                    qTr = qTrs[c]
                else:
                    qTr = qpool.tile([128, 4, 512], bf16, tag="qTr", name=f"qTr{c}")
                    qTrs[c] = qTr
                for m in ms:
                    pq = psum.tile([128, 512], f32, tag="mm512")
                    for db in range(16):
                        nc.tensor.matmul(
                            pq,
                            lhsT=wq_sb[:, m, db, :],
                            rhs=xT[:, db, tq0:tq0 + 512],
                            start=(db == 0),
                            stop=(db == 15),
                        )
                        if db % 2 == 1:
                            yield
                    rope(qTr[:, m, :], pq, c)

            def gen_kvproj(c):
                tq0 = c * 512
                pk = psum.tile([128, 512], f32, tag="mm512")
                for db in range(16):
                    nc.tensor.matmul(
                        pk,
                        lhsT=wkv_sb[:, db, 0:128],
                        rhs=xT[:, db, tq0:tq0 + 512],
                        start=(db == 0),
                        stop=(db == 15),
                    )
                    if db % 2 == 1:
                        yield
                rope(kTr_sb[:, tq0:tq0 + 512], pk, c)
                pv = psum.tile([128, 512], f32, tag="mm512")
                for db in range(16):
                    nc.tensor.matmul(
                        pv,
                        lhsT=wkv_sb[:, db, 128:256],
                        rhs=xT[:, db, tq0:tq0 + 512],
                        start=(db == 0),
                        stop=(db == 15),
                    )
                    if db % 2 == 1:
                        yield
                vch = vchp.tile([128, 512], bf16, tag="vch")
                nc.scalar.copy(out=vch, in_=pv)
                # per-block 2D DMA transposes: strided 3D-out DMAT corrupts
                # data in the XBAR path, so keep each out contiguous
                for rr in range(4):
                    nc.sync.dma_start_transpose(
                        out=v_sb[:, 4 * c + rr, 0:128],
                        in_=vch[:, rr * 128:(rr + 1) * 128],
                    )

            def run_gen(g):
                for _ in g:
                    pass

            # ---- projection prologue: everything attention chunk 3 needs ----
            run_gen(gen_qproj(3, ms=(0,)))
            run_gen(gen_kvproj(3))
            run_gen(gen_qproj(3, ms=(1, 2, 3)))
            run_gen(gen_kvproj(2))
            run_gen(gen_kvproj(1))
            run_gen(gen_kvproj(0))
            nc.scalar.dma_start(out=wc_sb, in_=wc_d)

            # ---- fillers: remaining projections, then out-projections ----
            out_pool_cm = [None]
            psum_o = [None]

            def gen_outproj(tq0, otT):
                """Generator: out-projection for chunk at tq0 from otT
                [128d, 4h, 512t]; yields after each psum tile (4 matmuls)."""
                for tb in range(4):
                    row = tq0 + tb * 128
                    for oc in range(4):
                        po = psum_o[0].tile([128, 512], f32, tag="out")
                        for h in range(4):
                            nc.tensor.matmul(
                                po,
                                lhsT=otT[:, h, tb, :],
                                rhs=wc_sb[:, h, oc * 512:(oc + 1) * 512],
                                start=(h == 0),
                                stop=(h == 3),
                            )
                        osb = osbp.tile([128, 512], f32, tag="osb")
                        nc.vector.tensor_copy(out=osb, in_=po)
                        nc.sync.dma_start(
                            out=out_d[row:row + 128, oc * 512:(oc + 1) * 512],
                            in_=osb,
                        )
                        yield

            proj_gens = [gen_qproj(2), gen_qproj(1), gen_qproj(0)]
            out_gens = []

            def pull_filler(n=1):
                """Emit up to n filler steps: projections first, then
                out-projections (swapping PSUM pools at the boundary)."""
                while n > 0:
                    if proj_gens:
                        try:
                            next(proj_gens[0])
                            n -= 1
                        except StopIteration:
                            proj_gens.pop(0)
                            if not proj_gens:
                                # all projections done: swap proj psum pool
                                # for the out-projection pool (2 banks each)
                                proj_pool_cm.__exit__(None, None, None)
                                cm = tc.tile_pool(
                                    name="psum_o", bufs=2, space="PSUM"
                                )
                                out_pool_cm[0] = cm
                                psum_o[0] = cm.__enter__()
                    elif out_gens:
                        try:
                            next(out_gens[0])
                            n -= 1
                        except StopIteration:
                            out_gens.pop(0)
                    else:
                        return

            # ---- attention chunks, biggest first ----
            pending_outproj = []
            for c in (3, 2, 1, 0):
                tq0 = c * 512
                qTr = qTrs[c]
                nkb = 4 * c + 4
                otT_c = otTp.tile([128, 4, 4, 128], bf16, tag="otT", name=f"otT{c}")
                for h in range(4):
                    if h == 1 and pending_outproj:
                        # release the previous chunk's out-projection only
                        # after one head of slack so its transposes land
                        out_gens.append(pending_outproj.pop(0))
                    otc = otcp.tile([128, 4, 128], bf16, tag="otc")
                    pav = [
                        psum_av.tile([128, 129], f32, tag="pav", name=f"pav{tb}")
                        for tb in range(4)
                    ]

                    def st_mm(kb):
                        """Score matmul for one key block (cols col0:512).

                        Diagonal blocks get the causal mask accumulated as a
                        second PE matmul (identity.T @ dmask == dmask), which
                        keeps the scores->exp chain entirely on the PE.
                        """
                        rr = kb - 4 * c
                        col0 = 0 if rr < 0 else 128 * rr
                        pst = psum_st.tile([128, 512], f32, tag="st")
                        nc.tensor.matmul(
                            pst[:, col0:512],
                            lhsT=kTr_sb[:, kb * 128:(kb + 1) * 128],
                            rhs=qTr[:, h, col0:512],
                            start=True,
                            stop=(rr < 0),
                            skip_group_check=True,
                        )
                        if rr >= 0:
                            nc.tensor.matmul(
                                pst[:, col0:col0 + 128],
                                lhsT=ident_sb,
                                rhs=dmask_sb,
                                start=False,
                                stop=True,
                                skip_group_check=True,
                            )
                        return pst, col0

                    # software-pipelined: emit S^T(kb+1) ahead of the
                    # exp-dependent AV matmuls of kb so the PE stream never
                    # head-blocks on the activation engine
                    pending = st_mm(0)
                    for kb in range(nkb):
                        pst, col0 = pending
                        if kb + 1 < nkb:
                            pending = st_mm(kb + 1)
                        e = epool.tile([128, 512], bf16, tag="E")
                        nc.scalar.activation(
                            out=e[:, col0:512], in_=pst[:, col0:512], func=EXP
                        )
                        rr = kb - 4 * c
                        tb_lo = rr if rr > 0 else 0
                        for tb in range(tb_lo, 4):
                            nc.tensor.matmul(
                                pav[tb],
                                lhsT=e[:, tb * 128:(tb + 1) * 128],
                                rhs=v_sb[:, kb, 0:129],
                                start=(kb == 0),
                                stop=(kb == 4 * c + tb),
                            )
                        pull_filler(1)

                    # normalize: per-token (per-partition) reciprocal of the
                    # accumulated 129th column, then one blocked
                    # DMA-transpose [t,d] -> [d,t] for the out-proj
                    for tb in range(4):
                        r = rpool.tile([128, 1], f32, tag="r")
                        nc.vector.reciprocal_approx_fast(
                            out=r, in_=pav[tb][:, 128:129]
                        )
                        if tb % 2 == 0:
                            nc.vector.tensor_scalar_mul(
                                otc[:, tb, :], pav[tb][:, 0:128], r
                            )
                        else:
                            nc.scalar.activation(
                                out=otc[:, tb, :], in_=pav[tb][:, 0:128],
                                func=COPY, scale=r,
                            )
                    nc.sync.dma_start_transpose(
                        out=otT_c[:, h, :, :], in_=otc[:, :, :]
                    )
                    pull_filler(2)

                # queue this chunk's out-projection as filler work (released
                # one head into the next chunk's attention)
                pending_outproj.append(gen_outproj(tq0, otT_c))
            while pending_outproj:
                out_gens.append(pending_outproj.pop(0))

            # drain remaining fillers (last chunk's outproj + leftovers)
            pull_filler(10**9)
            if out_pool_cm[0] is not None:
                out_pool_cm[0].__exit__(None, None, None)

    nc.compile()
    return nc


def _host_prep(x, Wq, Wkv, Wc):
    """Shard + relayout the full inputs into the 8 per-core input dicts."""
    import ml_dtypes

    bf = ml_dtypes.bfloat16
    dk, H, KV = DK, N_HEADS, N_KV_HEADS
    x = np.asarray(x, np.float32)
    Wq = np.asarray(Wq, np.float32)
    Wkv = np.asarray(Wkv, np.float32)
    Wc = np.asarray(Wc, np.float32)

    p = np.concatenate([np.arange(0, dk, 2), np.arange(1, dk, 2)])
    perm_q = np.concatenate([h * dk + p for h in range(H)])
    Wq_p = (Wq / math.sqrt(dk))[perm_q]
    perm_k = np.concatenate([g * dk + p for g in range(KV)])
    Wk_p = Wkv[:KV * dk][perm_k]
    Wv = Wkv[KV * dk:]

    pairs = np.arange(dk // 2, dtype=np.float64)
    freqs = 1.0 / (ROPE_THETA ** (2.0 * pairs / dk))
    ang = np.arange(S, dtype=np.float64)[:, None] * freqs[None, :]
    cos_t = np.cos(ang).astype(np.float32).T  # [64, S]
    sin_t = np.sin(ang).astype(np.float32).T
    c2 = np.ascontiguousarray(np.concatenate([cos_t, cos_t], 0))   # [128, S]
    ss = np.ascontiguousarray(np.concatenate([-sin_t, sin_t], 0))  # [128, S]

    jj = np.arange(128)[None, :]
    pp = np.arange(128)[:, None]
    dmask = np.where(pp <= jj, 0.0, NEG).astype(bf)
    ident = np.eye(128, dtype=bf)

    maps = []
    for core in range(NCORES):
        b, g = core // 4, core % 4
        # [p, m, db, 128]: feature-partition p, head m, contraction block db
        wq_l = np.ascontiguousarray(
            Wq_p[512 * g:512 * g + 512].T
            .reshape(16, 128, 4, 128).transpose(1, 2, 0, 3)
        ).astype(bf)
        wkv_sl = np.concatenate(
            [Wk_p[g * dk:(g + 1) * dk], Wv[g * dk:(g + 1) * dk]], 0
        ).T  # [2048, 256]
        wkv_l = np.ascontiguousarray(
            wkv_sl.reshape(16, 128, 256).transpose(1, 0, 2)
        ).astype(bf)
        wc_l = np.ascontiguousarray(
            Wc[:, 512 * g:512 * g + 512].T.reshape(4, 128, 2048).transpose(1, 0, 2)
        ).astype(bf)
        xt_l = np.ascontiguousarray(
            x[b].T.reshape(16, 128, S).transpose(1, 0, 2)
        ).astype(bf)
        maps.append(dict(
            x=xt_l, wq=wq_l, wkv=wkv_l, wc=wc_l,
            cos2=c2, ss=ss, dmask=dmask, ident=ident,
        ))
    return maps


def kernel(x, Wq, Wkv, Wc):
    global _COMPILED, _LAST_RESULT
    from concourse.bass_utils import run_bass_kernel_spmd

    if _COMPILED is None:
        _COMPILED = _build()
    in_maps = _host_prep(x, Wq, Wkv, Wc)
    res = run_bass_kernel_spmd(
        _COMPILED, in_maps, core_ids=list(range(NCORES)), trace=_TRACE
    )
    _LAST_RESULT = res
    outs = [res.results[i]["out"] for i in range(NCORES)]
    full = np.stack(
        [outs[0] + outs[1] + outs[2] + outs[3],
         outs[4] + outs[5] + outs[6] + outs[7]], 0
    ).astype(np.float32)
    return full
